# revision 1
# baseline (speedup 1.0000x reference)
"""Grouped-Query Attention (B=1, L=4096, D=1024, 16 q-heads, 4 kv-heads, hd=64)
on 8 Trainium2 NeuronCores.

Sharding: core c owns q-heads {2c, 2c+1} and their shared kv-head c//2.
Each core computes Q/K/V projections for its heads from the full (replicated)
x, runs dense softmax attention for its 2 heads, and produces a partial
output projection  attn_heads @ Wo[head_rows]  of full shape [4096, 1024].
Host sums the 8 partials and adds bo (row-parallel all-reduce on host).

Per-core dataflow (bf16 on the PE array, fp32 PSUM):
  x^T [1024,4096] (host-pretransposed bf16) -> SBUF
  K^T duplicated on partitions 0-63 / 64-127 so the two heads' score
  matmuls (C=64) pack into disjoint PE row-groups and run concurrently.
  Q^T [128, L]: head0 on partitions 0-63, head1 on 64-127 (pre-scaled 1/8).
  per 512-wide q-block, per k-group of 3 128-tiles, per head:
     S^T tile [128k, 512q] = (K^T slice).T @ Q^T_h       (PE, row-group packed)
     P^T = exp(S^T)                                      (ACT, 3 banks/instr)
     outT[65, 512] += (V-tile|ones).T @ P^T              (PE, C=128; row 64 = denom)
  epilogue: fast recip(denom), rank-1 broadcast matmul, DVE scale
  out[Lchunk, 1024] = sum_h attnT_h.T @ Wo_h             (PE, C=64 x2)
"""

import os

os.environ.setdefault("MYCRO_LOCAL_CACHE", "1")

import numpy as np
import ml_dtypes

import concourse.bass as bass
import concourse.bacc as bacc
import concourse.mybir as mybir
from concourse.tile import TileContext
from concourse.bass_utils import run_bass_kernel_spmd

BF16 = mybir.dt.bfloat16
F32 = mybir.dt.float32
AF = mybir.ActivationFunctionType

D = 1024
L = 4096
NHEAD = 16
NKV = 4
HD = 64
NCORES = 8
HPC = NHEAD // NCORES  # 2 q heads per core
QB = 512               # q-block width
NQB = L // QB          # 8
KT = 128               # k-tile
NKT = L // KT          # 32
KG = 3                 # k-tiles per exp group (3 PSUM banks)
NF = D // 128          # 8 feature chunks
SCALE = 0.125          # 1/sqrt(64)

_CACHE = {}


def _build(has_bias):
    nc = bacc.Bacc("TRN2", target_bir_lowering=False, debug=False)

    xT = nc.declare_dram_parameter("xT", [D, L], BF16, isOutput=False)
    wq = nc.declare_dram_parameter("wq", [D, HPC * HD], BF16, isOutput=False)
    wk = nc.declare_dram_parameter("wk", [D, HD], BF16, isOutput=False)
    wv = nc.declare_dram_parameter("wv", [D, HD], BF16, isOutput=False)
    wo0 = nc.declare_dram_parameter("wo0", [HD, D], BF16, isOutput=False)
    wo1 = nc.declare_dram_parameter("wo1", [HD, D], BF16, isOutput=False)
    bq = nc.declare_dram_parameter("bq", [1, HPC * HD], BF16, isOutput=False)
    bk = nc.declare_dram_parameter("bk", [1, HD], BF16, isOutput=False)
    bv = nc.declare_dram_parameter("bv", [1, HD], BF16, isOutput=False)
    out = nc.declare_dram_parameter("out", [L, D], F32, isOutput=True)

    with TileContext(nc) as tc:
        with (
            tc.tile_pool(name="sing", bufs=1) as sing,
            tc.tile_pool(name="ptp", bufs=3) as ptp,
            tc.tile_pool(name="attp", bufs=2) as attp,
            tc.tile_pool(name="nrm", bufs=3) as nrm,
            tc.tile_pool(name="obp", bufs=3) as obp,
            tc.tile_pool(name="psA", bufs=2, space="PSUM") as psA,
            tc.tile_pool(name="psB", bufs=2, space="PSUM") as psB,
        ):
            # ---- resident SBUF tensors ----
            xT_sb = sing.tile([128, NF, L], BF16)
            wq_sb = sing.tile([128, NF, HPC * HD], BF16)
            wk_sb = sing.tile([128, NF, HD], BF16)
            wv_sb = sing.tile([128, NF, HD], BF16)
            wo0_sb = sing.tile([HD, D], BF16)
            wo1_sb = sing.tile([HD, D], BF16)
            ones_f = sing.tile([65, HD], F32)       # fp32 ones (recip bcast, row 64)
            KT_sb = sing.tile([HD, L], BF16)
            QT_sb = sing.tile([HD, HPC, L], BF16)
            V_sb = sing.tile([128, NKT, HD + 1], BF16)  # col 64 = 1.0 (denom)
            if has_bias:
                bq_sb = sing.tile([1, HPC * HD], BF16)
                bk_sb = sing.tile([1, HD], BF16)
                bv_sb = sing.tile([1, HD], BF16)
                ones_b = sing.tile([1, QB], BF16)

            for f in range(NF):
                fs = slice(128 * f, 128 * (f + 1))
                nc.sync.dma_start(out=xT_sb[:, f, :], in_=xT[fs, :])
                nc.sync.dma_start(out=wq_sb[:, f, :], in_=wq[fs, :])
                nc.sync.dma_start(out=wk_sb[:, f, :], in_=wk[fs, :])
                nc.sync.dma_start(out=wv_sb[:, f, :], in_=wv[fs, :])
            nc.sync.dma_start(out=wo0_sb, in_=wo0[:, :])
            nc.sync.dma_start(out=wo1_sb, in_=wo1[:, :])
            if has_bias:
                nc.sync.dma_start(out=bq_sb, in_=bq[:, :])
                nc.sync.dma_start(out=bk_sb, in_=bk[:, :])
                nc.sync.dma_start(out=bv_sb, in_=bv[:, :])
                nc.gpsimd.memset(ones_b, 1.0)
            nc.gpsimd.memset(ones_f, 1.0)
            nc.gpsimd.memset(V_sb[:, :, HD], 1.0)

            # ---- projections ----
            # K^T[64, L] = Wk.T @ x^T (+ bk) on rows 0-63
            for n in range(NQB):
                ns = slice(QB * n, QB * (n + 1))
                kps = psA.tile([HD, QB], F32, tag="st")
                for f in range(NF):
                    nc.tensor.matmul(kps, wk_sb[:, f, :], xT_sb[:, f, ns],
                                     start=(f == 0),
                                     stop=(not has_bias and f == NF - 1))
                if has_bias:
                    nc.tensor.matmul(kps, bk_sb, ones_b, start=False, stop=True)
                nc.vector.tensor_copy(KT_sb[:, ns], kps)

            # V[L, 64] = x @ Wv (+ bv)   (natural layout, k on partitions)
            for l in range(NKT):
                ls = slice(KT * l, KT * (l + 1))
                vps = psA.tile([128, HD], F32, tag="st")
                for f in range(NF):
                    nc.tensor.matmul(vps, xT_sb[:, f, ls], wv_sb[:, f, :],
                                     start=(f == 0),
                                     stop=(not has_bias and f == NF - 1))
                if has_bias:
                    nc.tensor.matmul(vps, ones_b[:, 0:KT], bv_sb,
                                     start=False, stop=True)
                nc.vector.tensor_copy(V_sb[:, l, 0:HD], vps)

            # ---- per q-block: Q^T projection, then attention ----
            for q in range(NQB):
                qs = slice(QB * q, QB * (q + 1))

                # Q^T[128, qb] = (Wq.T @ x^T + bq) / 8, both heads stacked
                for h in range(HPC):
                    hs = slice(HD * h, HD * (h + 1))
                    qps = psA.tile([HD, QB], F32, tag="st", name="qps")
                    for f in range(NF):
                        nc.tensor.matmul(qps, wq_sb[:, f, hs], xT_sb[:, f, qs],
                                         start=(f == 0),
                                         stop=(not has_bias and f == NF - 1))
                    if has_bias:
                        nc.tensor.matmul(qps, bq_sb[:, hs], ones_b,
                                         start=False, stop=True)
                    nc.vector.tensor_scalar_mul(QT_sb[:, h, qs], qps, SCALE)

                atT = [attp.tile([HD, QB], BF16, tag=f"a{h}", name=f"atT{h}")
                       for h in range(HPC)]
                avps = [psB.tile([HD + 1, QB], F32, tag="av", name=f"avps{h}")
                        for h in range(HPC)]
                k = 0
                while k < NKT:
                    gs = min(KG, NKT - k)
                    stps = [psA.tile([128, KG, QB], F32, tag="st", name=f"stps{h}")
                            for h in range(HPC)]
                    ptsb = [ptp.tile([128, KG, QB], BF16, tag=f"pt{h}",
                                     name=f"ptsb{h}") for h in range(HPC)]
                    for j in range(gs):
                        ks = slice(KT * (k + j), KT * (k + j + 1))
                        nc.tensor.matmul(stps[0][:, j, :], KT_sb[:, ks],
                                         QT_sb[:, 0, qs], start=True, stop=True)
                        nc.tensor.matmul(stps[1][:, j, :], KT_sb[:, ks],
                                         QT_sb[:, 1, qs], start=True, stop=True)
                    for h in range(HPC):
                        nc.scalar.activation(ptsb[h][:, 0:gs, :],
                                             stps[h][:, 0:gs, :], AF.Exp)
                    for j in range(gs):
                        for h in range(HPC):
                            nc.tensor.matmul(avps[h], V_sb[:, k + j, :],
                                             ptsb[h][:, j, :],
                                             start=(k + j == 0),
                                             stop=(k + j == NKT - 1))
                    k += gs

                # epilogue per head: rows 0..63 /= row 64
                for h in range(HPC):
                    rd = nrm.tile([HD + 1, QB], F32, tag="rd")
                    nc.vector.tensor_copy(rd[HD:HD + 1, :], avps[h][HD:HD + 1, :])
                    nc.vector.reciprocal(rd[HD:HD + 1, :], rd[HD:HD + 1, :])
                    rbps = psA.tile([HD, QB], F32, tag="st", name="rbps")
                    nc.tensor.matmul(rbps, ones_f[HD:HD + 1, :], rd[HD:HD + 1, :],
                                     start=True, stop=True)
                    rbsb = nrm.tile([HD, QB], F32, tag="rb_sb")
                    nc.vector.tensor_copy(rbsb, rbps)
                    nc.vector.tensor_mul(atT[h], avps[h][0:HD, :], rbsb)

                # out[Lchunk, 1024] = attnT.T @ Wo  (two C=64 accumulating mms)
                for lc in range(QB // 128):
                    lcs = slice(128 * lc, 128 * (lc + 1))
                    ops = psA.tile([128, 2, QB], F32, tag="st", name="ops")
                    osb = obp.tile([128, D], F32, tag="ob")
                    for n in range(2):
                        ns = slice(QB * n, QB * (n + 1))
                        nc.tensor.matmul(ops[:, n, :], atT[0][:, lcs],
                                         wo0_sb[:, ns], start=True, stop=False)
                        nc.tensor.matmul(ops[:, n, :], atT[1][:, lcs],
                                         wo1_sb[:, ns], start=False, stop=True)
                    nc.vector.tensor_copy(osb, ops)
                    nc.sync.dma_start(
                        out=out[QB * q + 128 * lc:QB * q + 128 * (lc + 1), :],
                        in_=osb)
    nc.finalize()
    return nc


def _prep_inputs(x, Wq, bq, Wk, bk, Wv, bv, Wo, bo):
    bf = ml_dtypes.bfloat16
    xT = np.ascontiguousarray(np.asarray(x, dtype=np.float32)[0].T).astype(bf)
    Wq = np.asarray(Wq, dtype=np.float32)
    Wk = np.asarray(Wk, dtype=np.float32)
    Wv = np.asarray(Wv, dtype=np.float32)
    Wo = np.asarray(Wo, dtype=np.float32)
    bq = np.asarray(bq, dtype=np.float32)
    bk = np.asarray(bk, dtype=np.float32)
    bv = np.asarray(bv, dtype=np.float32)
    has_bias = bool(np.any(bq) or np.any(bk) or np.any(bv))
    in_maps = []
    for c in range(NCORES):
        qsl = slice(HPC * HD * c, HPC * HD * (c + 1))   # this core's q-head cols
        kv = c // 2                                     # its kv head
        ksl = slice(HD * kv, HD * (kv + 1))
        in_maps.append({
            "xT": xT,
            "wq": np.ascontiguousarray(Wq[:, qsl]).astype(bf),
            "wk": np.ascontiguousarray(Wk[:, ksl]).astype(bf),
            "wv": np.ascontiguousarray(Wv[:, ksl]).astype(bf),
            "wo0": np.ascontiguousarray(
                Wo[HPC * HD * c:HPC * HD * c + HD, :]).astype(bf),
            "wo1": np.ascontiguousarray(
                Wo[HPC * HD * c + HD:HPC * HD * (c + 1), :]).astype(bf),
            "bq": bq[qsl].reshape(1, -1).astype(bf),
            "bk": bk[ksl].reshape(1, -1).astype(bf),
            "bv": bv[ksl].reshape(1, -1).astype(bf),
        })
    return in_maps, has_bias


def run(inputs, trace=False):
    in_maps, has_bias = _prep_inputs(**inputs)
    key = ("nc", has_bias)
    if key not in _CACHE:
        _CACHE[key] = _build(has_bias)
    nc = _CACHE[key]
    res = run_bass_kernel_spmd(nc, in_maps, list(range(NCORES)), trace=trace)
    bo = np.asarray(inputs["bo"], dtype=np.float32)
    acc = np.zeros((L, D), dtype=np.float32)
    for r in res.results:
        acc += np.asarray(r["out"], dtype=np.float32)
    out = (acc + bo).reshape(1, L, D)
    return out, res


def kernel(**inputs):
    out, _ = run(inputs, trace=False)
    return out



# revision 41
# speedup vs baseline: 1.2790x; 1.2790x over previous
"""Grouped-Query Attention (B=1, L=4096, D=1024, 16 q-heads, 4 kv-heads, hd=64)
on 8 Trainium2 NeuronCores.

Sharding: core c owns q-heads {2c, 2c+1} and their shared kv-head c//2.
Each core computes Q/K/V projections for its heads from the full (replicated)
x, runs dense softmax attention for its 2 heads, and produces a partial
output projection  attn_heads @ Wo[head_rows]  of full shape [4096, 1024].
Host sums the 8 partials and adds bo (row-parallel all-reduce on host).

v2 pipeline (single-pass, PE-saturating):
  - K^T built with host-duplicated Wk so rows 0-63 and 64-127 both hold K^T;
    head-1 score matmuls then run with lhsT/rhs base partition 64 (PE
    row-quadrant 64) so Q^T can stay packed [128, qb] from one projection.
  - V via V^T projection (stream-bound) + PE transposes instead of the
    LDWEIGHTS-bound natural-layout projection.
  - Scores in 2-ktile groups [128,2,512] through a 3-slot PSUM ring; exp on
    ACT; PV lags scores per head (h0 by 2 groups, h1 by 7) so ACT latency
    and the epilogue reciprocal chain stay off the PE critical path.
  - k-tiles 16-31 run PV as fp8e4m3 DoubleRow matmuls (2 k-tiles per
    instruction; exp writes P in fp8, V pre-cast with the ones column in a
    zero-padded [128,2,128] stationary) — rel err ~1.5e-2 vs the 2e-2 gate,
    verified bit-exact against a numpy model of the same quantization.
  - Softmax denominator via the ones-column in the PV stationary (row 64);
    exact DVE reciprocal on the denom row (reciprocal_approx_fast corrupts
    unrelated state on HW), rank-1 fp32 broadcast matmul, DVE normalize
    into bf16 attn tiles, all deferred into the next q-block's early slots.
  - The first two score groups of each q-block are emitted inside the
    previous q-block's tail so ACT never starves across boundaries.
  - out[qb,1024] = sum_h attnT_h.T @ Wo_h, PSUM->SBUF on DVE, bf16 DMA out.
  - K/V/Q projections, transposes and out-proj are interleaved into the
    k-loops as PE filler; input DMA issue is spread across the sync/gpsimd/
    scalar queues with the first x^T quarter prioritized.
"""

import os

os.environ.setdefault("MYCRO_LOCAL_CACHE", "1")

import numpy as np
import ml_dtypes

import concourse.bass as bass
import concourse.bacc as bacc
import concourse.mybir as mybir
from concourse.tile import TileContext
from concourse.bass_utils import run_bass_kernel_spmd

BF16 = mybir.dt.bfloat16
F32 = mybir.dt.float32
F32R = mybir.dt.float32r
FP8 = mybir.dt.float8e4
AF = mybir.ActivationFunctionType
DR = mybir.MatmulPerfMode.DoubleRow

D = 1024
L = 4096
NHEAD = 16
NKV = 4
HD = 64
NCORES = 8
HPC = NHEAD // NCORES  # 2 q heads per core
QB = 512               # q-block width
NQB = L // QB          # 8
KT = 128               # k-tile
NKT = L // KT          # 32
NF = D // 128          # 8 feature chunks
NG = NKT // 2          # 16 score groups (2 ktiles each) per q-block

_CACHE = {}
USE_QUAD = os.environ.get("GQA_NO_QUAD", "") != "1"  # base-64 PE quadrant scores


def _build(has_bias):
    nc = bacc.Bacc("TRN2", target_bir_lowering=False, debug=False)

    xT = nc.declare_dram_parameter("xT", [D, L], BF16, isOutput=False)
    wq = nc.declare_dram_parameter("wq", [D, HPC * HD], BF16, isOutput=False)
    wkd = nc.declare_dram_parameter("wkd", [D, 2 * HD], BF16, isOutput=False)
    wv = nc.declare_dram_parameter("wv", [D, HD], BF16, isOutput=False)
    wo0 = nc.declare_dram_parameter("wo0", [HD, D], BF16, isOutput=False)
    wo1 = nc.declare_dram_parameter("wo1", [HD, D], BF16, isOutput=False)
    ident = nc.declare_dram_parameter("ident", [HD, HD], BF16, isOutput=False)
    if has_bias:
        bq = nc.declare_dram_parameter("bq", [1, HPC * HD], BF16, isOutput=False)
        bkd = nc.declare_dram_parameter("bkd", [1, 2 * HD], BF16, isOutput=False)
        bv = nc.declare_dram_parameter("bv", [1, HD], BF16, isOutput=False)
    out = nc.declare_dram_parameter("out", [L, D], BF16, isOutput=True)

    with TileContext(nc) as tc:
        with (
            tc.tile_pool(name="sing", bufs=1) as sing,
            tc.tile_pool(name="ptp", bufs=14) as ptp,
            tc.tile_pool(name="atp", bufs=2) as atp,
            tc.tile_pool(name="rcp", bufs=2) as rcp,
            tc.tile_pool(name="rbp", bufs=2) as rbp,
            tc.tile_pool(name="obp", bufs=2) as obp,
            tc.tile_pool(name="big", bufs=3, space="PSUM") as big,
            tc.tile_pool(name="avp", bufs=2, space="PSUM") as avp,
        ):
            # ---- resident SBUF tensors ----
            xT_sb = sing.tile([128, NF, L], BF16, name="xT_sb")
            wq_sb = sing.tile([128, NF, HPC * HD], BF16, name="wq_sb")
            wkd_sb = sing.tile([128, NF, 2 * HD], BF16, name="wkd_sb")
            wv_sb = sing.tile([128, NF, HD], BF16, name="wv_sb")
            wo0_sb = sing.tile([HD, D], BF16, name="wo0_sb")
            wo1_sb = sing.tile([HD, D], BF16, name="wo1_sb")
            ident_sb = sing.tile([HD, HD], BF16, name="ident_sb")
            KT_sb = sing.tile([128, L], BF16, name="KT_sb")
            VT_sb = sing.tile([HD, L], BF16, name="VT_sb")
            V_sb = sing.tile([128, NKT, HD + 1], BF16, name="V_sb")
            # fp8 copies of V (incl ones col) for the upper-half-k DoubleRow
            # PV matmuls; P there is quantized to e4m3 by the exp itself.
            F8G = 8  # first score group (2 ktiles each) computed in fp8
            V8_sb = sing.tile([128, max(1, NG - F8G), 2, 128], FP8,
                              name="V8_sb")
            if USE_QUAD:
                QT_sb = sing.tile([128, 2, QB], BF16, name="QT_sb")
            else:
                QT_sb = sing.tile([HD, 2, HPC, QB], BF16, name="QT_sb")
            ones_sb = sing.tile([HD + 1, HD], F32, name="ones_sb")
            if has_bias:
                bq_sb = sing.tile([1, HPC * HD], BF16, name="bq_sb")
                bkd_sb = sing.tile([1, 2 * HD], BF16, name="bkd_sb")
                bv_sb = sing.tile([1, HD], BF16, name="bv_sb")
                onesq = sing.tile([1, QB], BF16, name="onesq")

            # ---- weight / input DMAs ----
            # Issue cost is ~600ns per dma_start on the issuing engine's
            # sequencer; spread across idle queues and put the quarter-0
            # dependencies (wkd, xT q0) first so kproj starts early.
            nc.gpsimd.memset(ones_sb, 1.0)
            nc.gpsimd.memset(V8_sb[:, :, :, HD + 1:128], 0.0)
            nc.gpsimd.memset(V_sb[:, :, HD], 1.0)
            if has_bias:
                nc.gpsimd.memset(onesq, 1.0)

            def xq(eng, c, cs=None):
                cs = cs or slice(1024 * c, 1024 * (c + 1))
                for f in range(NF):
                    fs = slice(128 * f, 128 * (f + 1))
                    eng.dma_start(out=xT_sb[:, f, cs], in_=xT[fs, cs])

            for f in range(NF):
                fs = slice(128 * f, 128 * (f + 1))
                nc.sync.dma_start(out=wkd_sb[:, f, :], in_=wkd[fs, :])
                nc.gpsimd.dma_start(out=wv_sb[:, f, :], in_=wv[fs, :])
                nc.scalar.dma_start(out=wq_sb[:, f, :], in_=wq[fs, :])
            xq(nc.sync, 0, slice(0, 512))
            xq(nc.sync, 0, slice(512, 1024))
            nc.gpsimd.dma_start(out=ident_sb, in_=ident[:, :])
            xq(nc.gpsimd, 1)
            xq(nc.scalar, 2)
            nc.scalar.dma_start(out=wo0_sb, in_=wo0[:, :])
            nc.scalar.dma_start(out=wo1_sb, in_=wo1[:, :])
            xq(nc.sync, 3)
            if has_bias:
                nc.sync.dma_start(out=bq_sb, in_=bq[:, :])
                nc.sync.dma_start(out=bkd_sb, in_=bkd[:, :])
                nc.sync.dma_start(out=bv_sb, in_=bv[:, :])

            # ---- helper emitters (each = one PSUM "big" ring slot) ----
            def kproj(n):
                ns = slice(QB * n, QB * (n + 1))
                kps = big.tile([128, QB], F32, tag="big", name="kps")
                for f in range(NF):
                    nc.tensor.matmul(kps, wkd_sb[:, f, :], xT_sb[:, f, ns],
                                     start=(f == 0),
                                     stop=(not has_bias and f == NF - 1))
                if has_bias:
                    nc.tensor.matmul(kps, bkd_sb, onesq, start=False, stop=True)
                nc.vector.tensor_copy(KT_sb[:, ns], kps)

            def vtproj(n):
                ns = slice(QB * n, QB * (n + 1))
                vps = big.tile([128, QB], F32, tag="big", name="vps")
                for f in range(NF):
                    nc.tensor.matmul(vps[0:HD, :], wv_sb[:, f, :],
                                     xT_sb[:, f, ns], start=(f == 0),
                                     stop=(not has_bias and f == NF - 1))
                if has_bias:
                    nc.tensor.matmul(vps[0:HD, :], bv_sb, onesq,
                                     start=False, stop=True)
                nc.vector.tensor_copy(VT_sb[:, ns], vps[0:HD, :])

            def vtrans(kt0):  # transpose 2 ktiles of V^T -> V [128, kt, 64]
                tps = big.tile([128, 2, HD], BF16, tag="big", name="tps")
                for j in range(2):
                    ks = slice(KT * (kt0 + j), KT * (kt0 + j + 1))
                    nc.tensor.transpose(tps[:, j, :], VT_sb[:, ks], ident_sb)
                nc.vector.tensor_copy(V_sb[:, kt0:kt0 + 2, 0:HD], tps[:, 0:2, :])
                if kt0 >= 2 * F8G:
                    gi = (kt0 - 2 * F8G) // 2
                    nc.vector.tensor_copy(V8_sb[:, gi, :, 0:HD + 1],
                                          V_sb[:, kt0:kt0 + 2, :])

            def qproj(q):
                qs = slice(QB * q, QB * (q + 1))
                if USE_QUAD:
                    qps = big.tile([128, QB], F32, tag="big", name="qps")
                    for f in range(NF):
                        nc.tensor.matmul(qps, wq_sb[:, f, :], xT_sb[:, f, qs],
                                         start=(f == 0),
                                         stop=(not has_bias and f == NF - 1))
                    if has_bias:
                        nc.tensor.matmul(qps, bq_sb, onesq,
                                         start=False, stop=True)
                    nc.vector.tensor_copy(QT_sb[:, q % 2, :], qps)
                else:
                    qps = big.tile([128, HPC, QB], F32, tag="big", name="qps")
                    for h in range(HPC):
                        hs = slice(HD * h, HD * (h + 1))
                        for f in range(NF):
                            nc.tensor.matmul(qps[0:HD, h, :], wq_sb[:, f, hs],
                                             xT_sb[:, f, qs], start=(f == 0),
                                             stop=(not has_bias and f == NF - 1))
                        if has_bias:
                            nc.tensor.matmul(qps[0:HD, h, :], bq_sb[:, hs],
                                             onesq, start=False, stop=True)
                    nc.vector.tensor_copy(QT_sb[:, q % 2, :, :],
                                          qps[0:HD, :, :])

            def outproj(q, lc, atT):
                ops = big.tile([128, 2, QB], F32, tag="big", name="ops")
                lcs = slice(128 * lc, 128 * (lc + 1))
                for n in range(2):
                    ns = slice(QB * n, QB * (n + 1))
                    nc.tensor.matmul(ops[:, n, :], atT[0][:, lcs],
                                     wo0_sb[:, ns], start=True, stop=False)
                    nc.tensor.matmul(ops[:, n, :], atT[1][:, lcs],
                                     wo1_sb[:, ns], start=False, stop=True)
                osb = obp.tile([128, D], BF16, tag="ob", name="osb")
                nc.vector.tensor_copy(osb, ops)
                r0 = QB * q + 128 * lc
                nc.sync.dma_start(out=out[r0:r0 + 128, :], in_=osb)

            SEQ = os.environ.get("GQA_SEQ", "") == "1"
            # ---- prologue: quarter 0 projections + Q^T(qb0) ----
            kproj(0)
            kproj(1)
            vtproj(0)
            vtrans(0)
            vtrans(2)
            vtproj(1)
            vtrans(4)
            vtrans(6)
            if SEQ:
                for n in range(2, 8):
                    kproj(n)
                    vtproj(n)
                for kt0 in range(8, 32, 2):
                    vtrans(kt0)
            qproj(0)

            # misc PE filler scheduled into k-loop slots, per q-block
            misc = {q: {} for q in range(NQB)}
            if not SEQ:
                # qb0: project quarters 1-3 just ahead of their first use
                q0 = [
                    (2, lambda: kproj(2)), (3, lambda: vtproj(2)),
                    (3, lambda: vtrans(8)), (4, lambda: vtrans(10)),
                    (4, lambda: kproj(3)), (5, lambda: vtproj(3)),
                    (5, lambda: vtrans(12)), (6, lambda: vtrans(14)),
                    (6, lambda: kproj(4)), (7, lambda: vtproj(4)),
                    (7, lambda: vtrans(16)), (8, lambda: kproj(5)),
                    (8, lambda: vtrans(18)), (9, lambda: vtproj(5)),
                    (9, lambda: vtrans(20)), (10, lambda: kproj(6)),
                    (10, lambda: vtrans(22)), (11, lambda: vtproj(6)),
                    (11, lambda: vtrans(24)), (12, lambda: kproj(7)),
                    (12, lambda: vtrans(26)), (13, lambda: vtproj(7)),
                    (13, lambda: vtrans(28)), (14, lambda: vtrans(30)),
                    (15, lambda: qproj(1)),
                ]
                for slot, fn in q0:
                    misc[0].setdefault(slot, []).append(fn)

            LAG = (2, 7)  # PV lag (in score groups) per head

            def mk_fin(rc_t, av_t, at_t):
                # rank-1 broadcast of 1/denom + normalize; deferred into the
                # next q-block so the serial DVE reciprocal hides behind PE
                # work.
                def fin():
                    rbps = big.tile([128, QB], F32, tag="big", name="rbps")
                    nc.tensor.matmul(rbps[0:HD, :],
                                     ones_sb[HD:HD + 1, 0:HD],
                                     rc_t[HD:HD + 1, :],
                                     start=True, stop=True)
                    rbsb = rbp.tile([HD, QB], F32, tag="rb", name="rbsb")
                    nc.vector.tensor_copy(rbsb, rbps[0:HD, :])
                    nc.vector.tensor_mul(at_t, av_t[0:HD, :], rbsb)
                return fin

            prev_atT = None
            prev_fin = None
            carry = []  # score groups of q-block q pre-emitted in q-1's tail
            for q in range(NQB):
                qs_half = q % 2
                if SEQ and q >= 1:
                    qproj(q)
                if not SEQ:
                    if prev_fin:
                        for slot, fn in zip((1, 2, 3, 4), prev_fin):
                            misc[q].setdefault(slot, []).append(fn)
                    if q >= 1:
                        for i, lc in enumerate((0, 1, 2, 3)):
                            misc[q].setdefault(5 + 2 * i, []).append(
                                (lambda lc_=lc, q_=q, at_=prev_atT:
                                 outproj(q_ - 1, lc_, at_)))
                    if 1 <= q < NQB - 1:
                        misc[q].setdefault(13, []).append(
                            lambda q_=q: qproj(q_ + 1))

                avps = [avp.tile([128, QB], F32, tag="av", name=f"av{h}")
                        for h in range(HPC)]
                pend = {0: [], 1: []}
                ncarry = 0
                for g_, h_, pt_ in carry:
                    pend[h_].append((2 * g_, pt_))
                    ncarry = max(ncarry, g_ + 1)
                carry = []

                def pv_emit(h, pend=pend, avps=avps):
                    kt0, pt = pend[h].pop(0)
                    if kt0 >= 2 * F8G:
                        gi = (kt0 - 2 * F8G) // 2
                        nc.tensor.matmul(avps[h][0:128, :],
                                         V8_sb[:, gi, :, :],
                                         pt[:, 0:2, :],
                                         start=False, stop=(kt0 == NKT - 2),
                                         perf_mode=DR)
                    else:
                        for j in range(2):
                            nc.tensor.matmul(avps[h][0:HD + 1, :],
                                             V_sb[:, kt0 + j, :],
                                             pt[:, j, :],
                                             start=(kt0 + j == 0),
                                             stop=False)

                def sc_group(qhalf, g, h):
                    hs = slice(HD * h, HD * (h + 1))
                    stps = big.tile([128, 2, QB], F32, tag="big", name="stps")
                    pt = ptp.tile([128, 2, QB],
                                  FP8 if g >= F8G else BF16,
                                  tag="pt", name="pt")
                    for j in range(2):
                        kt = 2 * g + j
                        ks = slice(KT * kt, KT * (kt + 1))
                        if USE_QUAD:
                            qrhs = QT_sb[hs, qhalf, :]
                            klhs = KT_sb[hs, ks]
                        else:
                            qrhs = QT_sb[0:HD, qhalf, h, :]
                            klhs = KT_sb[0:HD, ks]
                        nc.tensor.matmul(stps[:, j, :], klhs, qrhs,
                                         start=True, stop=True)
                    nc.scalar.activation(pt[:, 0:2, :], stps[:, 0:2, :],
                                         AF.Exp)
                    return pt

                for g in range(NG):
                    if g >= ncarry:
                        for h in range(HPC):
                            pend[h].append((2 * g, sc_group(qs_half, g, h)))
                    for h in range(HPC):
                        if g >= LAG[h]:
                            pv_emit(h)
                    for fn in misc[q].get(g, ()):
                        fn()

                # tail: drain h0, start its reciprocal, then interleave the
                # next q-block's first score groups with the h1 drain so ACT
                # never starves across the boundary
                atT = [atp.tile([HD, QB], BF16, tag=f"a{h}", name=f"atT{h}")
                       for h in range(HPC)]
                while pend[0]:
                    pv_emit(0)
                rc0 = rcp.tile([HD + 1, QB], F32, tag="rcp", name="rc0")
                nc.vector.reciprocal(rc0[HD:HD + 1, :], avps[0][HD:HD + 1, :])
                fin0 = mk_fin(rc0, avps[0], atT[0])
                rc1 = rcp.tile([HD + 1, QB], F32, tag="rcp", name="rc1")

                def recip1(rc1=rc1, av1=avps[1]):
                    nc.vector.reciprocal(rc1[HD:HD + 1, :],
                                         av1[HD:HD + 1, :])

                def recip1a(rc1=rc1, av1=avps[1]):
                    nc.vector.reciprocal(rc1[HD:HD + 1, 0:QB // 2],
                                         av1[HD:HD + 1, 0:QB // 2])

                def recip1b(rc1=rc1, av1=avps[1]):
                    nc.vector.reciprocal(rc1[HD:HD + 1, QB // 2:QB],
                                         av1[HD:HD + 1, QB // 2:QB])

                fin1 = mk_fin(rc1, avps[1], atT[1])
                if SEQ:
                    while pend[1]:
                        pv_emit(1)
                    fin0()
                    recip1()
                    fin1()
                    prev_fin = None
                elif q < NQB - 1:
                    # interleave the next q-block's first score groups with
                    # the h1 PV drain so ACT never starves at the boundary
                    for gg, hh in ((0, 0), (0, 1), (1, 0), (1, 1)):
                        carry.append((gg, hh,
                                      sc_group((q + 1) % 2, gg, hh)))
                        for _ in range(2):
                            if pend[1]:
                                pv_emit(1)
                    while pend[1]:
                        pv_emit(1)
                    prev_fin = [fin0, recip1a, recip1b, fin1]
                else:
                    # final tail: overlap h0's out-proj half with h1's
                    # reciprocal/normalize
                    for _ in range(4):
                        if pend[1]:
                            pv_emit(1)
                    fin0()
                    while pend[1]:
                        pv_emit(1)
                    recip1()
                    opst = []
                    for lc in (0, 1):
                        lcs = slice(128 * lc, 128 * (lc + 1))
                        ops = big.tile([128, 2, QB], F32, tag="big",
                                       name="ops")
                        for n in range(2):
                            ns = slice(QB * n, QB * (n + 1))
                            nc.tensor.matmul(ops[:, n, :], atT[0][:, lcs],
                                             wo0_sb[:, ns],
                                             start=True, stop=False)
                        opst.append(ops)
                    fin1()
                    for lc in (0, 1):
                        lcs = slice(128 * lc, 128 * (lc + 1))
                        ops = opst[lc]
                        for n in range(2):
                            ns = slice(QB * n, QB * (n + 1))
                            nc.tensor.matmul(ops[:, n, :], atT[1][:, lcs],
                                             wo1_sb[:, ns],
                                             start=False, stop=True)
                        osb = obp.tile([128, D], BF16, tag="ob", name="osb")
                        nc.vector.tensor_copy(osb, ops)
                        r0 = QB * q + 128 * lc
                        nc.sync.dma_start(out=out[r0:r0 + 128, :], in_=osb)
                    for lc in (2, 3):
                        outproj(q, lc, atT)
                    prev_fin = None
                prev_atT = atT
                if SEQ:
                    for lc in range(4):
                        outproj(q, lc, atT)
    nc.finalize()
    return nc


def _prep_inputs(x, Wq, bq, Wk, bk, Wv, bv, Wo, bo):
    bf = ml_dtypes.bfloat16
    xT = np.ascontiguousarray(np.asarray(x, dtype=np.float32)[0].T).astype(bf)
    Wq = np.asarray(Wq, dtype=np.float32) * 0.125  # fold 1/sqrt(hd)
    Wk = np.asarray(Wk, dtype=np.float32)
    Wv = np.asarray(Wv, dtype=np.float32)
    Wo = np.asarray(Wo, dtype=np.float32)
    bq = np.asarray(bq, dtype=np.float32) * 0.125
    bk = np.asarray(bk, dtype=np.float32)
    bv = np.asarray(bv, dtype=np.float32)
    ident = np.eye(HD, dtype=np.float32).astype(bf)
    has_bias = bool(np.any(bq) or np.any(bk) or np.any(bv))
    in_maps = []
    for c in range(NCORES):
        qsl = slice(HPC * HD * c, HPC * HD * (c + 1))   # this core's q-head cols
        kv = c // 2                                     # its kv head
        ksl = slice(HD * kv, HD * (kv + 1))
        wk_c = np.ascontiguousarray(Wk[:, ksl])
        bk_c = bk[ksl]
        im = {
            "xT": xT,
            "wq": np.ascontiguousarray(Wq[:, qsl]).astype(bf),
            "wkd": np.concatenate([wk_c, wk_c], axis=1).astype(bf),
            "wv": np.ascontiguousarray(Wv[:, ksl]).astype(bf),
            "wo0": np.ascontiguousarray(
                Wo[HPC * HD * c:HPC * HD * c + HD, :]).astype(bf),
            "wo1": np.ascontiguousarray(
                Wo[HPC * HD * c + HD:HPC * HD * (c + 1), :]).astype(bf),
            "ident": ident,
        }
        if has_bias:
            bk_cd = np.concatenate([bk_c, bk_c])
            im["bq"] = bq[qsl].reshape(1, -1).astype(bf)
            im["bkd"] = bk_cd.reshape(1, -1).astype(bf)
            im["bv"] = bv[ksl].reshape(1, -1).astype(bf)
        in_maps.append(im)
    return in_maps, has_bias


def run(inputs, trace=False):
    in_maps, has_bias = _prep_inputs(**inputs)
    key = ("nc", has_bias)
    if key not in _CACHE:
        _CACHE[key] = _build(has_bias)
    nc = _CACHE[key]
    res = run_bass_kernel_spmd(nc, in_maps, list(range(NCORES)), trace=trace)
    bo = np.asarray(inputs["bo"], dtype=np.float32)
    acc = np.zeros((L, D), dtype=np.float32)
    for r in res.results:
        acc += np.asarray(r["out"], dtype=np.float32)
    out = (acc + bo).reshape(1, L, D)
    return out, res


def kernel(**inputs):
    out, _ = run(inputs, trace=False)
    return out


# revision 42
# speedup vs baseline: 1.2982x; 1.0150x over previous
"""Grouped-Query Attention (B=1, L=4096, D=1024, 16 q-heads, 4 kv-heads, hd=64)
on 8 Trainium2 NeuronCores.

Sharding: core c owns q-heads {2c, 2c+1} and their shared kv-head c//2.
Each core computes Q/K/V projections for its heads from the full (replicated)
x, runs dense softmax attention for its 2 heads, and produces a partial
output projection  attn_heads @ Wo[head_rows]  of full shape [4096, 1024].
Host sums the 8 partials and adds bo (row-parallel all-reduce on host).

v2 pipeline (single-pass, PE-saturating):
  - K^T built with host-duplicated Wk so rows 0-63 and 64-127 both hold K^T;
    head-1 score matmuls then run with lhsT/rhs base partition 64 (PE
    row-quadrant 64) so Q^T can stay packed [128, qb] from one projection.
  - V via V^T projection (stream-bound) + PE transposes instead of the
    LDWEIGHTS-bound natural-layout projection.
  - Scores in 2-ktile groups [128,2,512] through a 3-slot PSUM ring; exp on
    ACT; PV lags scores per head (h0 by 2 groups, h1 by 7) so ACT latency
    and the epilogue reciprocal chain stay off the PE critical path.
  - k-tiles 16-31 run PV as fp8e4m3 DoubleRow matmuls (2 k-tiles per
    instruction; exp writes P in fp8, V pre-cast with the ones column in a
    zero-padded [128,2,128] stationary) — rel err ~1.5e-2 vs the 2e-2 gate,
    verified bit-exact against a numpy model of the same quantization.
  - Softmax denominator via the ones-column in the PV stationary (row 64);
    exact DVE reciprocal on the denom row (reciprocal_approx_fast corrupts
    unrelated state on HW), rank-1 fp32 broadcast matmul, DVE normalize
    into bf16 attn tiles, all deferred into the next q-block's early slots.
  - The first two score groups of each q-block are emitted inside the
    previous q-block's tail so ACT never starves across boundaries.
  - out[qb,1024] = sum_h attnT_h.T @ Wo_h, PSUM->SBUF on DVE, bf16 DMA out.
  - K/V/Q projections, transposes and out-proj are interleaved into the
    k-loops as PE filler; input DMA issue is spread across the sync/gpsimd/
    scalar queues with the first x^T quarter prioritized.
"""

import os

os.environ.setdefault("MYCRO_LOCAL_CACHE", "1")

import numpy as np
import ml_dtypes

import concourse.bass as bass
import concourse.bacc as bacc
import concourse.mybir as mybir
from concourse.tile import TileContext
from concourse.bass_utils import run_bass_kernel_spmd

BF16 = mybir.dt.bfloat16
F32 = mybir.dt.float32
F32R = mybir.dt.float32r
FP8 = mybir.dt.float8e4
AF = mybir.ActivationFunctionType
DR = mybir.MatmulPerfMode.DoubleRow

D = 1024
L = 4096
NHEAD = 16
NKV = 4
HD = 64
NCORES = 8
HPC = NHEAD // NCORES  # 2 q heads per core
QB = 512               # q-block width
NQB = L // QB          # 8
KT = 128               # k-tile
NKT = L // KT          # 32
NF = D // 128          # 8 feature chunks
NG = NKT // 2          # 16 score groups (2 ktiles each) per q-block

_CACHE = {}
USE_QUAD = os.environ.get("GQA_NO_QUAD", "") != "1"  # base-64 PE quadrant scores


def _build(has_bias):
    nc = bacc.Bacc("TRN2", target_bir_lowering=False, debug=False)

    xT = nc.declare_dram_parameter("xT", [D, L], BF16, isOutput=False)
    wq = nc.declare_dram_parameter("wq", [D, HPC * HD], BF16, isOutput=False)
    wkd = nc.declare_dram_parameter("wkd", [D, 2 * HD], BF16, isOutput=False)
    wv = nc.declare_dram_parameter("wv", [D, HD], BF16, isOutput=False)
    wo0 = nc.declare_dram_parameter("wo0", [HD, D], BF16, isOutput=False)
    wo1 = nc.declare_dram_parameter("wo1", [HD, D], BF16, isOutput=False)
    ident = nc.declare_dram_parameter("ident", [HD, HD], BF16, isOutput=False)
    if has_bias:
        bq = nc.declare_dram_parameter("bq", [1, HPC * HD], BF16, isOutput=False)
        bkd = nc.declare_dram_parameter("bkd", [1, 2 * HD], BF16, isOutput=False)
        bv = nc.declare_dram_parameter("bv", [1, HD], BF16, isOutput=False)
    out = nc.declare_dram_parameter("out", [L, D], BF16, isOutput=True)

    with TileContext(nc) as tc:
        with (
            tc.tile_pool(name="sing", bufs=1) as sing,
            tc.tile_pool(name="ptp", bufs=14) as ptp,
            tc.tile_pool(name="atp", bufs=2) as atp,
            tc.tile_pool(name="rcp", bufs=2) as rcp,
            tc.tile_pool(name="rbp", bufs=2) as rbp,
            tc.tile_pool(name="obp", bufs=2) as obp,
            tc.tile_pool(name="big", bufs=3, space="PSUM") as big,
            tc.tile_pool(name="avp", bufs=2, space="PSUM") as avp,
        ):
            # ---- resident SBUF tensors ----
            xT_sb = sing.tile([128, NF, L], BF16, name="xT_sb")
            wq_sb = sing.tile([128, NF, HPC * HD], BF16, name="wq_sb")
            wkd_sb = sing.tile([128, NF, 2 * HD], BF16, name="wkd_sb")
            wv_sb = sing.tile([128, NF, HD], BF16, name="wv_sb")
            wo0_sb = sing.tile([HD, D], BF16, name="wo0_sb")
            wo1_sb = sing.tile([HD, D], BF16, name="wo1_sb")
            ident_sb = sing.tile([HD, HD], BF16, name="ident_sb")
            KT_sb = sing.tile([128, L], BF16, name="KT_sb")
            VT_sb = sing.tile([HD, L], BF16, name="VT_sb")
            V_sb = sing.tile([128, NKT, HD + 1], BF16, name="V_sb")
            # fp8 copies of V (incl ones col) for the upper-half-k DoubleRow
            # PV matmuls; P there is quantized to e4m3 by the exp itself.
            F8G = 8  # first score group (2 ktiles each) computed in fp8
            V8_sb = sing.tile([128, max(1, NG - F8G), 2, 128], FP8,
                              name="V8_sb")
            if USE_QUAD:
                QT_sb = sing.tile([128, 2, QB], BF16, name="QT_sb")
            else:
                QT_sb = sing.tile([HD, 2, HPC, QB], BF16, name="QT_sb")
            ones_sb = sing.tile([HD + 1, HD], F32, name="ones_sb")
            if has_bias:
                bq_sb = sing.tile([1, HPC * HD], BF16, name="bq_sb")
                bkd_sb = sing.tile([1, 2 * HD], BF16, name="bkd_sb")
                bv_sb = sing.tile([1, HD], BF16, name="bv_sb")
                onesq = sing.tile([1, QB], BF16, name="onesq")

            # ---- weight / input DMAs ----
            # Issue cost is ~600ns per dma_start on the issuing engine's
            # sequencer; spread across idle queues and put the quarter-0
            # dependencies (wkd, xT q0) first so kproj starts early.
            nc.gpsimd.memset(ones_sb, 1.0)
            nc.gpsimd.memset(V8_sb[:, :, :, HD + 1:128], 0.0)
            nc.gpsimd.memset(V_sb[:, :, HD], 1.0)
            if has_bias:
                nc.gpsimd.memset(onesq, 1.0)

            def xq(eng, c, cs=None):
                cs = cs or slice(1024 * c, 1024 * (c + 1))
                for f in range(NF):
                    fs = slice(128 * f, 128 * (f + 1))
                    eng.dma_start(out=xT_sb[:, f, cs], in_=xT[fs, cs])

            # first x^T block on the scalar queue in parallel with wkd on
            # sync, so kproj(0) can start ~7us in instead of ~17us
            xq(nc.scalar, 0, slice(0, 512))
            for f in range(NF):
                fs = slice(128 * f, 128 * (f + 1))
                nc.sync.dma_start(out=wkd_sb[:, f, :], in_=wkd[fs, :])
                nc.gpsimd.dma_start(out=wv_sb[:, f, :], in_=wv[fs, :])
                nc.scalar.dma_start(out=wq_sb[:, f, :], in_=wq[fs, :])
            xq(nc.sync, 0, slice(512, 1024))
            nc.gpsimd.dma_start(out=ident_sb, in_=ident[:, :])
            xq(nc.gpsimd, 1)
            xq(nc.scalar, 2)
            nc.scalar.dma_start(out=wo0_sb, in_=wo0[:, :])
            nc.scalar.dma_start(out=wo1_sb, in_=wo1[:, :])
            xq(nc.sync, 3)
            if has_bias:
                nc.sync.dma_start(out=bq_sb, in_=bq[:, :])
                nc.sync.dma_start(out=bkd_sb, in_=bkd[:, :])
                nc.sync.dma_start(out=bv_sb, in_=bv[:, :])

            # ---- helper emitters (each = one PSUM "big" ring slot) ----
            def kproj(n):
                ns = slice(QB * n, QB * (n + 1))
                kps = big.tile([128, QB], F32, tag="big", name="kps")
                for f in range(NF):
                    nc.tensor.matmul(kps, wkd_sb[:, f, :], xT_sb[:, f, ns],
                                     start=(f == 0),
                                     stop=(not has_bias and f == NF - 1))
                if has_bias:
                    nc.tensor.matmul(kps, bkd_sb, onesq, start=False, stop=True)
                nc.vector.tensor_copy(KT_sb[:, ns], kps)

            def vtproj(n):
                ns = slice(QB * n, QB * (n + 1))
                vps = big.tile([128, QB], F32, tag="big", name="vps")
                for f in range(NF):
                    nc.tensor.matmul(vps[0:HD, :], wv_sb[:, f, :],
                                     xT_sb[:, f, ns], start=(f == 0),
                                     stop=(not has_bias and f == NF - 1))
                if has_bias:
                    nc.tensor.matmul(vps[0:HD, :], bv_sb, onesq,
                                     start=False, stop=True)
                nc.vector.tensor_copy(VT_sb[:, ns], vps[0:HD, :])

            def vtrans(kt0):  # transpose 2 ktiles of V^T -> V [128, kt, 64]
                tps = big.tile([128, 2, HD], BF16, tag="big", name="tps")
                for j in range(2):
                    ks = slice(KT * (kt0 + j), KT * (kt0 + j + 1))
                    nc.tensor.transpose(tps[:, j, :], VT_sb[:, ks], ident_sb)
                nc.vector.tensor_copy(V_sb[:, kt0:kt0 + 2, 0:HD], tps[:, 0:2, :])
                if kt0 >= 2 * F8G:
                    gi = (kt0 - 2 * F8G) // 2
                    nc.vector.tensor_copy(V8_sb[:, gi, :, 0:HD + 1],
                                          V_sb[:, kt0:kt0 + 2, :])

            def qproj(q):
                qs = slice(QB * q, QB * (q + 1))
                if USE_QUAD:
                    qps = big.tile([128, QB], F32, tag="big", name="qps")
                    for f in range(NF):
                        nc.tensor.matmul(qps, wq_sb[:, f, :], xT_sb[:, f, qs],
                                         start=(f == 0),
                                         stop=(not has_bias and f == NF - 1))
                    if has_bias:
                        nc.tensor.matmul(qps, bq_sb, onesq,
                                         start=False, stop=True)
                    nc.vector.tensor_copy(QT_sb[:, q % 2, :], qps)
                else:
                    qps = big.tile([128, HPC, QB], F32, tag="big", name="qps")
                    for h in range(HPC):
                        hs = slice(HD * h, HD * (h + 1))
                        for f in range(NF):
                            nc.tensor.matmul(qps[0:HD, h, :], wq_sb[:, f, hs],
                                             xT_sb[:, f, qs], start=(f == 0),
                                             stop=(not has_bias and f == NF - 1))
                        if has_bias:
                            nc.tensor.matmul(qps[0:HD, h, :], bq_sb[:, hs],
                                             onesq, start=False, stop=True)
                    nc.vector.tensor_copy(QT_sb[:, q % 2, :, :],
                                          qps[0:HD, :, :])

            def outproj(q, lc, atT):
                ops = big.tile([128, 2, QB], F32, tag="big", name="ops")
                lcs = slice(128 * lc, 128 * (lc + 1))
                for n in range(2):
                    ns = slice(QB * n, QB * (n + 1))
                    nc.tensor.matmul(ops[:, n, :], atT[0][:, lcs],
                                     wo0_sb[:, ns], start=True, stop=False)
                    nc.tensor.matmul(ops[:, n, :], atT[1][:, lcs],
                                     wo1_sb[:, ns], start=False, stop=True)
                osb = obp.tile([128, D], BF16, tag="ob", name="osb")
                nc.vector.tensor_copy(osb, ops)
                r0 = QB * q + 128 * lc
                nc.sync.dma_start(out=out[r0:r0 + 128, :], in_=osb)

            SEQ = os.environ.get("GQA_SEQ", "") == "1"
            # ---- prologue: quarter 0 projections + Q^T(qb0) ----
            kproj(0)
            kproj(1)
            vtproj(0)
            vtrans(0)
            vtrans(2)
            vtproj(1)
            vtrans(4)
            vtrans(6)
            if SEQ:
                for n in range(2, 8):
                    kproj(n)
                    vtproj(n)
                for kt0 in range(8, 32, 2):
                    vtrans(kt0)
            qproj(0)

            # misc PE filler scheduled into k-loop slots, per q-block
            misc = {q: {} for q in range(NQB)}
            if not SEQ:
                # qb0: project quarters 1-3 just ahead of their first use
                q0 = [
                    (2, lambda: kproj(2)), (3, lambda: vtproj(2)),
                    (3, lambda: vtrans(8)), (4, lambda: vtrans(10)),
                    (4, lambda: kproj(3)), (5, lambda: vtproj(3)),
                    (5, lambda: vtrans(12)), (6, lambda: vtrans(14)),
                    (6, lambda: kproj(4)), (7, lambda: vtproj(4)),
                    (7, lambda: vtrans(16)), (8, lambda: kproj(5)),
                    (8, lambda: vtrans(18)), (9, lambda: vtproj(5)),
                    (9, lambda: vtrans(20)), (10, lambda: kproj(6)),
                    (10, lambda: vtrans(22)), (11, lambda: vtproj(6)),
                    (11, lambda: vtrans(24)), (12, lambda: kproj(7)),
                    (12, lambda: vtrans(26)), (13, lambda: vtproj(7)),
                    (13, lambda: vtrans(28)), (14, lambda: vtrans(30)),
                    (15, lambda: qproj(1)),
                ]
                for slot, fn in q0:
                    misc[0].setdefault(slot, []).append(fn)

            LAG = (2, 7)  # PV lag (in score groups) per head

            def mk_fin(rc_t, av_t, at_t):
                # rank-1 broadcast of 1/denom + normalize; deferred into the
                # next q-block so the serial DVE reciprocal hides behind PE
                # work.
                def fin():
                    rbps = big.tile([128, QB], F32, tag="big", name="rbps")
                    nc.tensor.matmul(rbps[0:HD, :],
                                     ones_sb[HD:HD + 1, 0:HD],
                                     rc_t[HD:HD + 1, :],
                                     start=True, stop=True)
                    rbsb = rbp.tile([HD, QB], F32, tag="rb", name="rbsb")
                    nc.vector.tensor_copy(rbsb, rbps[0:HD, :])
                    nc.vector.tensor_mul(at_t, av_t[0:HD, :], rbsb)
                return fin

            prev_atT = None
            prev_fin = None
            carry = []  # score groups of q-block q pre-emitted in q-1's tail
            for q in range(NQB):
                qs_half = q % 2
                if SEQ and q >= 1:
                    qproj(q)
                if not SEQ:
                    if prev_fin:
                        for slot, fn in zip((1, 2, 3, 4), prev_fin):
                            misc[q].setdefault(slot, []).append(fn)
                    if q >= 1:
                        for i, lc in enumerate((0, 1, 2, 3)):
                            misc[q].setdefault(5 + 2 * i, []).append(
                                (lambda lc_=lc, q_=q, at_=prev_atT:
                                 outproj(q_ - 1, lc_, at_)))
                    if 1 <= q < NQB - 1:
                        misc[q].setdefault(13, []).append(
                            lambda q_=q: qproj(q_ + 1))

                avps = [avp.tile([128, QB], F32, tag="av", name=f"av{h}")
                        for h in range(HPC)]
                pend = {0: [], 1: []}
                ncarry = 0
                for g_, h_, pt_ in carry:
                    pend[h_].append((2 * g_, pt_))
                    ncarry = max(ncarry, g_ + 1)
                carry = []

                def pv_emit(h, pend=pend, avps=avps):
                    kt0, pt = pend[h].pop(0)
                    if kt0 >= 2 * F8G:
                        gi = (kt0 - 2 * F8G) // 2
                        nc.tensor.matmul(avps[h][0:128, :],
                                         V8_sb[:, gi, :, :],
                                         pt[:, 0:2, :],
                                         start=False, stop=(kt0 == NKT - 2),
                                         perf_mode=DR)
                    else:
                        for j in range(2):
                            nc.tensor.matmul(avps[h][0:HD + 1, :],
                                             V_sb[:, kt0 + j, :],
                                             pt[:, j, :],
                                             start=(kt0 + j == 0),
                                             stop=False)

                def sc_group(qhalf, g, h):
                    hs = slice(HD * h, HD * (h + 1))
                    stps = big.tile([128, 2, QB], F32, tag="big", name="stps")
                    pt = ptp.tile([128, 2, QB],
                                  FP8 if g >= F8G else BF16,
                                  tag="pt", name="pt")
                    for j in range(2):
                        kt = 2 * g + j
                        ks = slice(KT * kt, KT * (kt + 1))
                        if USE_QUAD:
                            qrhs = QT_sb[hs, qhalf, :]
                            klhs = KT_sb[hs, ks]
                        else:
                            qrhs = QT_sb[0:HD, qhalf, h, :]
                            klhs = KT_sb[0:HD, ks]
                        nc.tensor.matmul(stps[:, j, :], klhs, qrhs,
                                         start=True, stop=True)
                    nc.scalar.activation(pt[:, 0:2, :], stps[:, 0:2, :],
                                         AF.Exp)
                    return pt

                for g in range(NG):
                    if g >= ncarry:
                        for h in range(HPC):
                            pend[h].append((2 * g, sc_group(qs_half, g, h)))
                    for h in range(HPC):
                        if g >= LAG[h]:
                            pv_emit(h)
                    for fn in misc[q].get(g, ()):
                        fn()

                # tail: drain h0, start its reciprocal, then interleave the
                # next q-block's first score groups with the h1 drain so ACT
                # never starves across the boundary
                atT = [atp.tile([HD, QB], BF16, tag=f"a{h}", name=f"atT{h}")
                       for h in range(HPC)]
                while pend[0]:
                    pv_emit(0)
                rc0 = rcp.tile([HD + 1, QB], F32, tag="rcp", name="rc0")
                nc.vector.reciprocal(rc0[HD:HD + 1, :], avps[0][HD:HD + 1, :])
                fin0 = mk_fin(rc0, avps[0], atT[0])
                rc1 = rcp.tile([HD + 1, QB], F32, tag="rcp", name="rc1")

                def recip1(rc1=rc1, av1=avps[1]):
                    nc.vector.reciprocal(rc1[HD:HD + 1, :],
                                         av1[HD:HD + 1, :])

                def recip1a(rc1=rc1, av1=avps[1]):
                    nc.vector.reciprocal(rc1[HD:HD + 1, 0:QB // 2],
                                         av1[HD:HD + 1, 0:QB // 2])

                def recip1b(rc1=rc1, av1=avps[1]):
                    nc.vector.reciprocal(rc1[HD:HD + 1, QB // 2:QB],
                                         av1[HD:HD + 1, QB // 2:QB])

                fin1 = mk_fin(rc1, avps[1], atT[1])
                if SEQ:
                    while pend[1]:
                        pv_emit(1)
                    fin0()
                    recip1()
                    fin1()
                    prev_fin = None
                elif q < NQB - 1:
                    # interleave the next q-block's first score groups with
                    # the h1 PV drain so ACT never starves at the boundary
                    for gg, hh in ((0, 0), (0, 1), (1, 0), (1, 1)):
                        carry.append((gg, hh,
                                      sc_group((q + 1) % 2, gg, hh)))
                        for _ in range(2):
                            if pend[1]:
                                pv_emit(1)
                    while pend[1]:
                        pv_emit(1)
                    prev_fin = [fin0, recip1a, recip1b, fin1]
                else:
                    # final tail: overlap h0's out-proj half with h1's
                    # reciprocal/normalize
                    for _ in range(4):
                        if pend[1]:
                            pv_emit(1)
                    fin0()
                    while pend[1]:
                        pv_emit(1)
                    recip1()
                    opst = []
                    for lc in (0, 1):
                        lcs = slice(128 * lc, 128 * (lc + 1))
                        ops = big.tile([128, 2, QB], F32, tag="big",
                                       name="ops")
                        for n in range(2):
                            ns = slice(QB * n, QB * (n + 1))
                            nc.tensor.matmul(ops[:, n, :], atT[0][:, lcs],
                                             wo0_sb[:, ns],
                                             start=True, stop=False)
                        opst.append(ops)
                    fin1()
                    for lc in (0, 1):
                        lcs = slice(128 * lc, 128 * (lc + 1))
                        ops = opst[lc]
                        for n in range(2):
                            ns = slice(QB * n, QB * (n + 1))
                            nc.tensor.matmul(ops[:, n, :], atT[1][:, lcs],
                                             wo1_sb[:, ns],
                                             start=False, stop=True)
                        osb = obp.tile([128, D], BF16, tag="ob", name="osb")
                        nc.vector.tensor_copy(osb, ops)
                        r0 = QB * q + 128 * lc
                        nc.sync.dma_start(out=out[r0:r0 + 128, :], in_=osb)
                    for lc in (2, 3):
                        outproj(q, lc, atT)
                    prev_fin = None
                prev_atT = atT
                if SEQ:
                    for lc in range(4):
                        outproj(q, lc, atT)
    nc.finalize()
    return nc


def _prep_inputs(x, Wq, bq, Wk, bk, Wv, bv, Wo, bo):
    bf = ml_dtypes.bfloat16
    xT = np.ascontiguousarray(np.asarray(x, dtype=np.float32)[0].T).astype(bf)
    Wq = np.asarray(Wq, dtype=np.float32) * 0.125  # fold 1/sqrt(hd)
    Wk = np.asarray(Wk, dtype=np.float32)
    Wv = np.asarray(Wv, dtype=np.float32)
    Wo = np.asarray(Wo, dtype=np.float32)
    bq = np.asarray(bq, dtype=np.float32) * 0.125
    bk = np.asarray(bk, dtype=np.float32)
    bv = np.asarray(bv, dtype=np.float32)
    ident = np.eye(HD, dtype=np.float32).astype(bf)
    has_bias = bool(np.any(bq) or np.any(bk) or np.any(bv))
    in_maps = []
    for c in range(NCORES):
        qsl = slice(HPC * HD * c, HPC * HD * (c + 1))   # this core's q-head cols
        kv = c // 2                                     # its kv head
        ksl = slice(HD * kv, HD * (kv + 1))
        wk_c = np.ascontiguousarray(Wk[:, ksl])
        bk_c = bk[ksl]
        im = {
            "xT": xT,
            "wq": np.ascontiguousarray(Wq[:, qsl]).astype(bf),
            "wkd": np.concatenate([wk_c, wk_c], axis=1).astype(bf),
            "wv": np.ascontiguousarray(Wv[:, ksl]).astype(bf),
            "wo0": np.ascontiguousarray(
                Wo[HPC * HD * c:HPC * HD * c + HD, :]).astype(bf),
            "wo1": np.ascontiguousarray(
                Wo[HPC * HD * c + HD:HPC * HD * (c + 1), :]).astype(bf),
            "ident": ident,
        }
        if has_bias:
            bk_cd = np.concatenate([bk_c, bk_c])
            im["bq"] = bq[qsl].reshape(1, -1).astype(bf)
            im["bkd"] = bk_cd.reshape(1, -1).astype(bf)
            im["bv"] = bv[ksl].reshape(1, -1).astype(bf)
        in_maps.append(im)
    return in_maps, has_bias


def run(inputs, trace=False):
    in_maps, has_bias = _prep_inputs(**inputs)
    key = ("nc", has_bias)
    if key not in _CACHE:
        _CACHE[key] = _build(has_bias)
    nc = _CACHE[key]
    res = run_bass_kernel_spmd(nc, in_maps, list(range(NCORES)), trace=trace)
    bo = np.asarray(inputs["bo"], dtype=np.float32)
    acc = np.zeros((L, D), dtype=np.float32)
    for r in res.results:
        acc += np.asarray(r["out"], dtype=np.float32)
    out = (acc + bo).reshape(1, L, D)
    return out, res


def kernel(**inputs):
    out, _ = run(inputs, trace=False)
    return out
